# revision 52
# baseline (speedup 1.0000x reference)
"""MultiHeadAttention + RoPE kernel for 8 Trainium2 NeuronCores.

Sharding: core c in 0..7 -> batch b = c//4, head-group hg = c%4 (4 heads
each).  Each core computes its 4 heads' attention for its batch and a
partial output projection y_part = out_heads @ wo[head rows]; the host
sums the 4 partials per batch and adds bo.

Fully pipelined bf16 schedule (~199us/core in the cost model vs 255us
for the f32r baseline):
  - all matmul operands bf16 (same 1 cyc/row as f32r but half the DMA and
    SBUF); inputs stream over two DMA queues in consumption order so the
    first Q-projection matmul starts ~4us in
  - head-dims of wq/wk (and bq/bk/cos/sin) are host-permuted to interleave
    RoPE pairs (d, d+32) adjacently; Q.K is invariant under the shared
    permutation, and rotate-half becomes a DVE stream_shuffle (adjacent
    lane swap) + two bf16 muls + add, with the rotation sign folded into
    the sin table -- no PE rot matmul, no PSUM bank for it
  - Q/K projections run k-outer across sets of 3 PSUM groups so the first
    pass over x is paced by the chunk DMAs; PSUM evacuations are issued
    ahead of the cos/sin-dependent rope combines on the DVE FIFO
  - V natural [S, depth], evacuated by ACT copies (idle in phase A).  Its
    bias is separable through the O-projection ((attn+bv)@wo = attn@wo +
    bv@wo) and added exactly on the host.  A ones column makes PV
    accumulate the softmax denominator in PSUM row 64
  - phase B per (q-block, head): 12/16 key tiles exp'd on ACT (exact),
    4/16 on DVE via a Schraudolph fast-exp (bits = s*(0.125*128/ln2) + B,
    bitcast uint16->bf16, ~1.9% rms) to keep the exp stream ahead of PE;
    fast-exp tiles go through two [128,512] halves of the shared "yring"
    PSUM tag and their PV matmuls are deferred to the end of the head's
    accumulation (commutative), so they never stall the ACT exp ring
  - PV PSUM is evacuated raw in one DVE copy (frees the bank early); the
    denominator is plain-copied, gpsimd-partition-broadcast, and only
    then reciprocal'd (a broadcast reading a custom-DVE op's output races
    on hardware), all off the critical path
  - O-projection of q-block 0 is interleaved between q-block 1's heads;
    the final o-proj tail alternates PSUM slots with the idle exp ring
    and splits evacuation between DVE and ACT; y partials stored bf16
"""

import numpy as np
import ml_dtypes

import concourse.bacc as bacc
import concourse.mybir as mybir
from concourse.tile import TileContext

try:  # persistent XLA compile cache: repeat processes skip the ~4min compile
    import jax as _jax
    _jax.config.update("jax_compilation_cache_dir", "/tmp/jax_comp_cache")
    _jax.config.update("jax_persistent_cache_min_compile_time_secs", 1.0)
except Exception:
    pass

B, S, DM, H, DH = 2, 2048, 1024, 16, 64
NCORES = 8
HL = 4                 # heads per core
DHL = HL * DH          # 256
KCH = DM // 128        # 8 k-chunks of the model-dim contraction
SKT = S // 128         # 16 key tiles
QB = 1024              # q block
NQB = S // QB          # 2
QT_TILES = DHL // 128  # 2 m-tiles for the Q/K projections
VW = 66                # padded V row width (65 used: 64 dims + ones col)

F32 = mybir.dt.float32
BF16 = mybir.dt.bfloat16
U16 = mybir.dt.uint16
EXP = mybir.ActivationFunctionType.Exp
COPY = mybir.ActivationFunctionType.Copy
ADD = mybir.AluOpType.add
MULT = mybir.AluOpType.mult

# adjacent-pair swap within each 32-lane quadrant (RoPE rotate-half after
# the host head-dim interleave permutation)
SWAP_MASK = [j ^ 1 for j in range(32)]

# gpsimd fast-exp (Schraudolph on bf16 bits): exp(s/8) ~= bitcast_u16_bf16(
#   s * (0.125*128/ln2) + (16256 - 7 + 0.5) )   [~1.9% rms, zero-mean]
SCHRAUD_SKS = (2, 6, 10, 14)
SCH_A = float(0.125 * 128.0 / np.log(2.0))
SCH_B = 16256.0 - 7.0 + 0.5
SCHRAUD_ENGINE = "vector"   # gpsimd cannot read PSUM on HW

_CACHE = {}


def _build_nc():
    nc = bacc.Bacc()
    xT = nc.dram_tensor("xT", [DM, S], BF16, kind="ExternalInput")
    wq = nc.dram_tensor("wq", [DM, DHL], BF16, kind="ExternalInput")
    wk = nc.dram_tensor("wk", [DM, DHL], BF16, kind="ExternalInput")
    wv = nc.dram_tensor("wv", [DM, DHL], BF16, kind="ExternalInput")
    wo = nc.dram_tensor("wo", [128, QT_TILES, DM], BF16, kind="ExternalInput")
    bq = nc.dram_tensor("bq", [128, QT_TILES], F32, kind="ExternalInput")
    bk = nc.dram_tensor("bk", [128, QT_TILES], F32, kind="ExternalInput")
    cosT = nc.dram_tensor("cosT", [128, S], BF16, kind="ExternalInput")
    sinT = nc.dram_tensor("sinT", [128, S], BF16, kind="ExternalInput")
    y = nc.dram_tensor("y", [S, DM], BF16, kind="ExternalOutput")

    with TileContext(nc) as tc:
        with (
            tc.tile_pool(name="p0", bufs=1) as p0,
            tc.tile_pool(name="pt", bufs=2) as pt,
            tc.tile_pool(name="pb_exp", bufs=4) as pb_exp,
            tc.tile_pool(name="pb_n", bufs=2) as pb_n,
            tc.tile_pool(name="pc_y", bufs=2) as pc_y,
        ):
            qrope_r = p0.tile([128, QT_TILES, S], BF16)
            krope_r = p0.tile([128, QT_TILES, S], BF16)
            v_r = p0.tile([128, SKT, HL, VW], BF16)
            xT_r = p0.tile([128, KCH, S], BF16)
            wq_r = p0.tile([128, KCH, DHL], BF16)
            wk_r = p0.tile([128, KCH, DHL], BF16)
            wv_r = p0.tile([128, KCH, DHL], BF16)
            wo_r = p0.tile([128, QT_TILES, DM], BF16)
            outT_r = p0.tile([128, QT_TILES, S], BF16)
            cos_sb = p0.tile([128, S], BF16)
            sin_sb = p0.tile([128, S], BF16)
            bq_sb = p0.tile([128, QT_TILES], F32)
            bk_sb = p0.tile([128, QT_TILES], F32)

            # xT + w on the sync queue in consumption order (the shared DMA
            # device serializes transfers); tiny biases on the scalar queue
            nc.sync.dma_start(wq_r[:], wq.rearrange("(k p) n -> p k n", p=128))
            # x0 split in half so the very first matmul starts sooner
            nc.sync.dma_start(xT_r[:, 0, 0:QB], xT[0:128, 0:QB])
            nc.sync.dma_start(xT_r[:, 0, QB:S], xT[0:128, QB:S])
            for k in range(1, KCH):
                nc.sync.dma_start(xT_r[:, k, :], xT[k * 128:(k + 1) * 128, :])
            nc.sync.dma_start(wk_r[:], wk.rearrange("(k p) n -> p k n", p=128))
            nc.sync.dma_start(cos_sb[:], cosT[:, :])
            nc.sync.dma_start(sin_sb[:], sinT[:, :])
            nc.sync.dma_start(wv_r[:], wv.rearrange("(k p) n -> p k n", p=128))
            nc.sync.dma_start(wo_r[:], wo[:, :, :])
            nc.scalar.dma_start(bq_sb[:], bq[:, :])
            nc.scalar.dma_start(bk_sb[:], bk[:, :])

            ones_col = p0.tile([128, 1], BF16)
            nc.vector.memset(ones_col[:], 1.0)
            nc.vector.tensor_copy(
                v_r[:, :, :, DH:DH + 1],
                ones_col[:, None, None, :].broadcast_to([128, SKT, HL, 1]))
            # preload the exp ACT table while ACT is idle in phase A
            warm_in = p0.tile([1, 128], F32)
            warm = p0.tile([1, 128], F32)
            nc.vector.memset(warm_in[:], 1.0)
            nc.scalar.activation(warm[:], warm_in[:], EXP, scale=0.125)

            # ================= PHASE A =================
            with tc.tile_pool(name="ps_a", bufs=1, space="PSUM") as ps_a:
                # proj groups: (which weights, mt, q-block) -> rope dest
                groups = []
                for w_r, b_sb, dest in ((wq_r, bq_sb, qrope_r),
                                        (wk_r, bk_sb, krope_r)):
                    for mt in range(QT_TILES):
                        for qb_i in range(NQB):
                            groups.append((w_r, b_sb, dest, mt, qb_i))
                # sets of 3 share a k-outer pass; set 1 is all-wq (wk hasn't
                # arrived yet) and is paced by the x chunk DMAs
                order = [0, 1, 2, 3, 4, 5, 6, 7]
                sets = [order[0:3], order[3:6], order[6:8]]

                qb_by_group = {}
                for gset in sets:
                    pss = [ps_a.tile([128, QB], F32, tag="qkps", bufs=3,
                                     name=f"qkps{gi}") for gi in gset]
                    for k in range(KCH):
                        for ps, gi in zip(pss, gset):
                            _, _, _, mt, qb_i = groups[gi]
                            w_r = groups[gi][0]
                            q0 = qb_i * QB
                            for nq in range(QB // 512):
                                nc.tensor.matmul(
                                    ps[:, nq * 512:(nq + 1) * 512],
                                    w_r[:, k, mt * 128:(mt + 1) * 128],
                                    xT_r[:, k,
                                         q0 + nq * 512:q0 + (nq + 1) * 512],
                                    start=(k == 0), stop=(k == KCH - 1))
                    # only the PSUM evacuations go on the DVE FIFO here: they
                    # recycle the qkps ring for the next set with nothing
                    # (cos/sin, Pool) in front of them
                    for ps, gi in zip(pss, gset):
                        _, b_sb, _, mt, _ = groups[gi]
                        qb_r = pt.tile([128, QB], BF16, tag="qbr", bufs=8,
                                       name=f"qbr{gi}")
                        nc.vector.tensor_scalar(
                            out=qb_r[:], in0=ps[:],
                            scalar1=b_sb[:, mt:mt + 1],
                            scalar2=None, op0=ADD)
                        qb_by_group[gi] = qb_r

                # V projection: natural [S, depth]; bias-add evacuation on the
                # idle Pool engine so the DVE FIFO never gates the vps ring
                for sk in range(SKT):
                    vps = ps_a.tile([128, 512], F32, tag="vps", bufs=2)
                    for k in range(KCH):
                        nc.tensor.matmul(
                            vps[:, 0:DHL],
                            xT_r[:, k, sk * 128:(sk + 1) * 128],
                            wv_r[:, k, :],
                            start=(k == 0), stop=(k == KCH - 1))
                    nc.scalar.activation(
                        v_r[:, sk, :, 0:DH],
                        vps[:, 0:DHL].rearrange("p (h d) -> p h d", h=HL),
                        COPY)

                # rope combines, mt0 groups first (they gate phase B head 0)
                for gi in (0, 1, 4, 5, 2, 3, 6, 7):
                    _, _, dest, mt, qb_i = groups[gi]
                    q0 = qb_i * QB
                    qb_r = qb_by_group[gi]
                    rot = pt.tile([128, QB], BF16, tag="rot", bufs=2,
                                  name=f"rot{gi}")
                    nc.vector.stream_shuffle(rot[:], qb_r[:], SWAP_MASK)
                    t2 = pt.tile([128, QB], BF16, tag="t2", bufs=2,
                                 name=f"t2_{gi}")
                    nc.vector.tensor_mul(t2[:], rot[:], sin_sb[:, q0:q0 + QB])
                    t1 = pt.tile([128, QB], BF16, tag="t1", bufs=2,
                                 name=f"t1_{gi}")
                    nc.vector.tensor_mul(t1[:], qb_r[:],
                                         cos_sb[:, q0:q0 + QB])
                    nc.vector.tensor_add(dest[:, mt, q0:q0 + QB],
                                         t1[:], t2[:])

            # ================= PHASE B =================
            with tc.tile_pool(name="ps_b", bufs=1, space="PSUM") as ps_b:

                def oproj(qt, tail):
                    y_sb = pc_y.tile([128, DM], BF16, tag="ysb", bufs=6,
                                     name=f"ysb{qt}")
                    for ch in range(2):
                        ytag = "stps" if (tail and ch == 1) else "yring"
                        y_ps = ps_b.tile([128, 512], F32, tag=ytag,
                                         bufs=2, name=f"yps{qt}_{ch}")
                        for kc in range(QT_TILES):
                            nc.tensor.matmul(
                                y_ps[:],
                                outT_r[:, kc, qt * 128:(qt + 1) * 128],
                                wo_r[:, kc, ch * 512:(ch + 1) * 512],
                                start=(kc == 0), stop=(kc == QT_TILES - 1))
                        if tail and ch == 1:
                            nc.scalar.activation(
                                y_sb[:, ch * 512:(ch + 1) * 512], y_ps[:],
                                COPY)
                        else:
                            nc.vector.tensor_copy(
                                y_sb[:, ch * 512:(ch + 1) * 512], y_ps[:])
                    nc.sync.dma_start(y[qt * 128:(qt + 1) * 128, :], y_sb[:])

                def process_head(qb_i, h, q0, qbw, oproj_post,
                                 oproj_mid, last):
                    nqb = qbw // 512
                    mt = h // 2
                    half = (h % 2) * DH
                    qt_h = qrope_r[half:half + DH, mt, :]
                    kt_h = krope_r[half:half + DH, mt, :]
                    pv_ps = ps_b.tile([DH + 1, QB], F32, tag="pvps",
                                      bufs=1, name=f"pv{qb_i}_{h}_{q0}")
                    deferred = []
                    first_pv = [True] * nqb

                    def pv_mm(exp_slice, sk, nq, last_mm):
                        nc.tensor.matmul(
                            pv_ps[:, nq * 512:(nq + 1) * 512],
                            v_r[:, sk, h, 0:DH + 1],
                            exp_slice,
                            start=first_pv[nq], stop=last_mm)
                        first_pv[nq] = False

                    for sk in range(SKT):
                        if sk in SCHRAUD_SKS:
                            for nq in range(nqb):
                                sh_ps = ps_b.tile(
                                    [128, 512], F32, tag="yring", bufs=2,
                                    name=f"shps{sk}_{nq}")
                                nc.tensor.matmul(
                                    sh_ps[:],
                                    kt_h[:, sk * 128:(sk + 1) * 128],
                                    qt_h[:, q0 + nq * 512:
                                         q0 + (nq + 1) * 512],
                                    start=True, stop=True)
                                ei = pb_exp.tile([128, 512], U16,
                                                 tag="expi", bufs=6,
                                                 name=f"ei{sk}_{nq}")
                                nc.vector.tensor_scalar(
                                    out=ei[:], in0=sh_ps[:],
                                    scalar1=SCH_A, scalar2=SCH_B,
                                    op0=MULT, op1=ADD)
                                deferred.append(
                                    (ei[:, :].bitcast(BF16), sk, nq))
                        else:
                            st_ps = ps_b.tile([128, QB], F32, tag="stps",
                                              bufs=2)
                            for nq in range(nqb):
                                nc.tensor.matmul(
                                    st_ps[:, nq * 512:(nq + 1) * 512],
                                    kt_h[:, sk * 128:(sk + 1) * 128],
                                    qt_h[:, q0 + nq * 512:
                                         q0 + (nq + 1) * 512],
                                    start=True, stop=True)
                            ef = pb_exp.tile([128, QB], BF16,
                                             tag="expst")
                            nc.scalar.activation(ef[:, 0:qbw],
                                                 st_ps[:, 0:qbw], EXP,
                                                 scale=0.125)
                            for nq in range(nqb):
                                pv_mm(ef[:, nq * 512:(nq + 1) * 512],
                                      sk, nq, False)
                        for qt in oproj_mid.get(sk, ()):
                            oproj(qt, tail=False)
                    last_i = {}
                    for i, (_, _, nq) in enumerate(deferred):
                        last_i[nq] = i
                    for i, (exp_slice, sk, nq) in enumerate(deferred):
                        pv_mm(exp_slice, sk, nq, i == last_i[nq])

                    # evacuate PV raw (frees the bank in one copy);
                    # normalize off the critical path; the very last
                    # half normalizes straight from PSUM
                    if last:
                        pv_src = pv_ps
                    else:
                        pv_sb = pb_n.tile([DH + 1, QB], F32, tag="pvsb")
                        nc.vector.tensor_copy(pv_sb[:, 0:qbw],
                                              pv_ps[:, 0:qbw])
                        pv_src = pv_sb
                    den_t = pb_n.tile([1, QB], F32, tag="dent")
                    nc.vector.tensor_copy(den_t[:, 0:qbw],
                                          pv_src[DH:DH + 1, 0:qbw])
                    rec_b = pb_n.tile([DH, QB], F32, tag="recb")
                    nc.gpsimd.partition_broadcast(rec_b[:, 0:qbw],
                                                  den_t[:, 0:qbw])
                    nc.vector.reciprocal_approx_fast(
                        out=rec_b[:, 0:qbw], in_=rec_b[:, 0:qbw])
                    nc.vector.tensor_mul(
                        outT_r[half:half + DH, mt, q0:q0 + qbw],
                        pv_src[0:DH, 0:qbw], rec_b[:, 0:qbw])
                    for qt in oproj_post:
                        oproj(qt, tail=False)

                # qb0 heads, then qb1 heads with block-0 o-proj interleaved;
                # the last head is split into two 512-query halves so o-proj
                # tiles 8-11 hide inside the second half's compute
                for h in range(HL):
                    process_head(0, h, 0, QB, (), {}, False)
                process_head(1, 0, QB, QB, (0, 1), {}, False)
                process_head(1, 1, QB, QB, (2, 3), {}, False)
                process_head(1, 2, QB, QB, (4, 5), {}, False)
                process_head(1, 3, QB, 512, (6, 7), {}, False)
                process_head(1, 3, QB + 512, 512, (8, 9, 10, 11), {}, True)
                for qt in range(12, 16):
                    oproj(qt, tail=True)

    nc.finalize()
    return nc


# host-side head-dim interleave: new[2i] = old[i], new[2i+1] = old[i+32]
_PERM64 = np.empty(DH, dtype=np.int64)
_PERM64[0::2] = np.arange(32)
_PERM64[1::2] = np.arange(32) + 32


def _rope_tables():
    """cos'/sin'' tables in the permuted layout, [128, S] (2 heads stacked).
    cos'[2i] = cos'[2i+1] = cos(ang_i); sin''[2i] = -sin(ang_i),
    sin''[2i+1] = +sin(ang_i) (rotation sign folded in)."""
    inv_freq = 1.0 / (10000.0 ** (np.arange(0, DH, 2, dtype=np.float32) / DH))
    ang = np.arange(S, dtype=np.float32)[:, None] * inv_freq[None, :]  # [S,32]
    cosp = np.empty((DH, S), dtype=np.float32)
    sinp = np.empty((DH, S), dtype=np.float32)
    cosp[0::2] = cosp[1::2] = np.cos(ang).T
    sinp[0::2] = -np.sin(ang).T
    sinp[1::2] = np.sin(ang).T
    cosT = np.ascontiguousarray(np.vstack([cosp, cosp]))
    sinT = np.ascontiguousarray(np.vstack([sinp, sinp]))
    return (sinT.astype(ml_dtypes.bfloat16), cosT.astype(ml_dtypes.bfloat16))


def _make_runner(nc):
    """Build a cached jitted SPMD executor (mirrors the multi-core tail of
    concourse.bass2jax.run_bass_via_pjrt so repeat calls skip recompiles)."""
    import jax
    import numpy as _np
    from jax.sharding import Mesh, PartitionSpec
    from jax.experimental.shard_map import shard_map
    from concourse import bass2jax, mybir as _mybir

    bass2jax.install_neuronx_cc_hook()

    partition_name = (
        nc.partition_id_tensor.name if nc.partition_id_tensor else None)
    in_names, out_names, out_avals, zero_shapes = [], [], [], []
    for alloc in nc.m.functions[0].allocations:
        if not isinstance(alloc, _mybir.MemoryLocationSet):
            continue
        name = alloc.memorylocations[0].name
        if alloc.kind == "ExternalInput":
            if name != partition_name:
                in_names.append(name)
        elif alloc.kind == "ExternalOutput":
            out_names.append(name)
            shape = tuple(alloc.tensor_shape)
            dtype = _mybir.dt.np(alloc.dtype)
            out_avals.append(jax.core.ShapedArray(shape, dtype))
            zero_shapes.append((shape, dtype))
    n_params = len(in_names)
    all_names = in_names + out_names
    if partition_name is not None:
        all_names = all_names + [partition_name]

    def _body(*args):
        operands = list(args)
        if partition_name is not None:
            operands.append(bass2jax.partition_id_tensor())
        outs = bass2jax._bass_exec_p.bind(
            *operands,
            out_avals=tuple(out_avals),
            in_names=tuple(all_names),
            out_names=tuple(out_names),
            lowering_input_output_aliases=(),
            sim_require_finite=True,
            sim_require_nnan=True,
            nc=nc,
        )
        return tuple(outs)

    devices = jax.devices()[:NCORES]
    mesh = Mesh(_np.asarray(devices), ("core",))
    n_outs = len(out_names)
    sharded = jax.jit(
        shard_map(
            _body, mesh=mesh,
            in_specs=(PartitionSpec("core"),) * (n_params + n_outs),
            out_specs=(PartitionSpec("core"),) * n_outs,
            check_rep=False,
        ),
        donate_argnums=tuple(range(n_params, n_params + n_outs)),
        keep_unused=True,
    )

    def run(in_maps):
        concat_in = [
            _np.concatenate([_np.asarray(m[name]) for m in in_maps], axis=0)
            for name in in_names
        ]
        concat_zeros = [
            _np.zeros((NCORES * s[0], *s[1:]), dt) for (s, dt) in zero_shapes
        ]
        out_arrs = sharded(*concat_in, *concat_zeros)
        return [
            {
                name: _np.asarray(out_arrs[i]).reshape(
                    NCORES, *out_avals[i].shape)[c]
                for i, name in enumerate(out_names)
            }
            for c in range(NCORES)
        ]

    return run


def _get_runner():
    if "runner" not in _CACHE:
        nc = _build_nc()
        _CACHE["nc"] = nc
        _CACHE["runner"] = _make_runner(nc)
    return _CACHE["runner"]


def make_in_maps(x, wq, bq, wk, bk, wv, bv, wo, bo):
    """Build the 8 per-core input dicts from full inputs."""
    x = np.asarray(x, dtype=np.float32)
    if "tables" not in _CACHE:
        _CACHE["tables"] = _rope_tables()
    sinT, cosT = _CACHE["tables"]
    bf = ml_dtypes.bfloat16
    # head-dim interleave permutation applied per head to wq/wk/bq/bk
    permH = (np.arange(H)[:, None] * DH + _PERM64[None, :]).reshape(-1)
    wq_p = np.asarray(wq, np.float32)[:, permH]
    wk_p = np.asarray(wk, np.float32)[:, permH]
    bq_p = np.asarray(bq, np.float32)[permH]
    bk_p = np.asarray(bk, np.float32)[permH]
    in_maps = []
    for c in range(NCORES):
        b, hg = divmod(c, HL)
        sl = slice(hg * DHL, (hg + 1) * DHL)
        in_maps.append({
            "xT": np.ascontiguousarray(x[b].T).astype(bf),
            "wq": np.ascontiguousarray(wq_p[:, sl]).astype(bf),
            "wk": np.ascontiguousarray(wk_p[:, sl]).astype(bf),
            "wv": np.ascontiguousarray(
                np.asarray(wv, np.float32)[:, sl]).astype(bf),
            "wo": np.ascontiguousarray(
                np.asarray(wo, np.float32)[sl, :].reshape(QT_TILES, 128, DM)
                .transpose(1, 0, 2)).astype(bf),
            "bq": np.ascontiguousarray(bq_p[sl].reshape(QT_TILES, 128).T),
            "bk": np.ascontiguousarray(bk_p[sl].reshape(QT_TILES, 128).T),
            "cosT": cosT,
            "sinT": sinT,
        })
    return in_maps


def kernel(x, wq, bq, wk, bk, wv, bv, wo, bo):
    runner = _get_runner()
    in_maps = make_in_maps(x, wq, bq, wk, bk, wv, bv, wo, bo)
    results = runner(in_maps)
    bo = np.asarray(bo, dtype=np.float32)
    # V bias is separable through the O-projection: (attn+bv)@wo =
    # attn@wo + bv@wo, so the kernel skips it and we add it here exactly
    corr = (np.asarray(bv, np.float32) @ np.asarray(wo, np.float32)
            + bo).astype(np.float32)
    out = np.empty((B, S, DM), dtype=np.float32)
    for b in range(B):
        acc = results[b * HL + 0]["y"].astype(np.float32, copy=True)
        for hg in range(1, HL):
            acc += results[b * HL + hg]["y"]
        out[b] = acc + corr[None, :]
    return out


# revision 56
# speedup vs baseline: 1.0381x; 1.0381x over previous
"""MultiHeadAttention + RoPE kernel for 8 Trainium2 NeuronCores.

Sharding: core c in 0..7 -> batch b = c//4, head-group hg = c%4 (4 heads
each).  Each core computes its 4 heads' attention for its batch and a
partial output projection y_part = out_heads @ wo[head rows]; the host
sums the 4 partials per batch and adds bo.

Fully pipelined bf16 schedule (~199us/core in the cost model vs 255us
for the f32r baseline):
  - all matmul operands bf16 (same 1 cyc/row as f32r but half the DMA and
    SBUF); inputs stream over two DMA queues in consumption order so the
    first Q-projection matmul starts ~4us in
  - head-dims of wq/wk (and bq/bk/cos/sin) are host-permuted to interleave
    RoPE pairs (d, d+32) adjacently; Q.K is invariant under the shared
    permutation, and rotate-half becomes a DVE stream_shuffle (adjacent
    lane swap) + two bf16 muls + add, with the rotation sign folded into
    the sin table -- no PE rot matmul, no PSUM bank for it
  - Q/K projections run k-outer across sets of 3 PSUM groups so the first
    pass over x is paced by the chunk DMAs; PSUM evacuations are issued
    ahead of the cos/sin-dependent rope combines on the DVE FIFO
  - V natural [S, depth], evacuated by ACT copies (idle in phase A).  Its
    bias is separable through the O-projection ((attn+bv)@wo = attn@wo +
    bv@wo) and added exactly on the host.  A ones column makes PV
    accumulate the softmax denominator in PSUM row 64
  - phase B per (q-block, head): key tiles exp'd on ACT (exact) except
    4/16 on DVE via a Schraudolph fast-exp (bits = s*(0.125*128/ln2) + B,
    bitcast uint16->bf16, ~1.9% rms) -- 5/16 on q-block-0 heads, which
    have no o-proj PE filler and otherwise run ACT-coupled -- keeping the
    exp stream ahead of PE;
    fast-exp tiles go through two [128,512] halves of the shared "yring"
    PSUM tag and their PV matmuls are deferred to the end of the head's
    accumulation (commutative), so they never stall the ACT exp ring
  - PV PSUM is evacuated raw in one DVE copy (frees the bank early); the
    denominator is plain-copied, gpsimd-partition-broadcast, and only
    then reciprocal'd (a broadcast reading a custom-DVE op's output races
    on hardware), all off the critical path
  - O-projection of q-block 0 is interleaved between q-block 1's heads;
    the final o-proj tail alternates PSUM slots with the idle exp ring
    and splits evacuation between DVE and ACT; y partials stored bf16
"""

import numpy as np
import ml_dtypes

import concourse.bacc as bacc
import concourse.mybir as mybir
from concourse.tile import TileContext

try:  # persistent XLA compile cache: repeat processes skip the ~4min compile
    import jax as _jax
    _jax.config.update("jax_compilation_cache_dir", "/tmp/jax_comp_cache")
    _jax.config.update("jax_persistent_cache_min_compile_time_secs", 1.0)
except Exception:
    pass

B, S, DM, H, DH = 2, 2048, 1024, 16, 64
NCORES = 8
HL = 4                 # heads per core
DHL = HL * DH          # 256
KCH = DM // 128        # 8 k-chunks of the model-dim contraction
SKT = S // 128         # 16 key tiles
QB = 1024              # q block
NQB = S // QB          # 2
QT_TILES = DHL // 128  # 2 m-tiles for the Q/K projections
VW = 66                # padded V row width (65 used: 64 dims + ones col)

F32 = mybir.dt.float32
BF16 = mybir.dt.bfloat16
U16 = mybir.dt.uint16
EXP = mybir.ActivationFunctionType.Exp
COPY = mybir.ActivationFunctionType.Copy
ADD = mybir.AluOpType.add
MULT = mybir.AluOpType.mult

# adjacent-pair swap within each 32-lane quadrant (RoPE rotate-half after
# the host head-dim interleave permutation)
SWAP_MASK = [j ^ 1 for j in range(32)]

# gpsimd fast-exp (Schraudolph on bf16 bits): exp(s/8) ~= bitcast_u16_bf16(
#   s * (0.125*128/ln2) + (16256 - 7 + 0.5) )   [~1.9% rms, zero-mean]
SCHRAUD_SKS = (2, 6, 10, 14)
SCH_A = float(0.125 * 128.0 / np.log(2.0))
SCH_B = 16256.0 - 7.0 + 0.5
SCHRAUD_ENGINE = "vector"   # gpsimd cannot read PSUM on HW

_CACHE = {}


def _build_nc():
    nc = bacc.Bacc()
    xT = nc.dram_tensor("xT", [DM, S], BF16, kind="ExternalInput")
    wq = nc.dram_tensor("wq", [DM, DHL], BF16, kind="ExternalInput")
    wk = nc.dram_tensor("wk", [DM, DHL], BF16, kind="ExternalInput")
    wv = nc.dram_tensor("wv", [DM, DHL], BF16, kind="ExternalInput")
    wo = nc.dram_tensor("wo", [128, QT_TILES, DM], BF16, kind="ExternalInput")
    bq = nc.dram_tensor("bq", [128, QT_TILES], F32, kind="ExternalInput")
    bk = nc.dram_tensor("bk", [128, QT_TILES], F32, kind="ExternalInput")
    cosT = nc.dram_tensor("cosT", [128, S], BF16, kind="ExternalInput")
    sinT = nc.dram_tensor("sinT", [128, S], BF16, kind="ExternalInput")
    y = nc.dram_tensor("y", [S, DM], BF16, kind="ExternalOutput")

    with TileContext(nc) as tc:
        with (
            tc.tile_pool(name="p0", bufs=1) as p0,
            tc.tile_pool(name="pt", bufs=2) as pt,
            tc.tile_pool(name="pb_exp", bufs=4) as pb_exp,
            tc.tile_pool(name="pb_n", bufs=2) as pb_n,
            tc.tile_pool(name="pc_y", bufs=2) as pc_y,
        ):
            qrope_r = p0.tile([128, QT_TILES, S], BF16)
            krope_r = p0.tile([128, QT_TILES, S], BF16)
            v_r = p0.tile([128, SKT, HL, VW], BF16)
            xT_r = p0.tile([128, KCH, S], BF16)
            wq_r = p0.tile([128, KCH, DHL], BF16)
            wk_r = p0.tile([128, KCH, DHL], BF16)
            wv_r = p0.tile([128, KCH, DHL], BF16)
            wo_r = p0.tile([128, QT_TILES, DM], BF16)
            outT_r = p0.tile([128, QT_TILES, S], BF16)
            cos_sb = p0.tile([128, S], BF16)
            sin_sb = p0.tile([128, S], BF16)
            bq_sb = p0.tile([128, QT_TILES], F32)
            bk_sb = p0.tile([128, QT_TILES], F32)

            # xT + w on the sync queue in consumption order (the shared DMA
            # device serializes transfers); tiny biases on the scalar queue
            nc.sync.dma_start(wq_r[:], wq.rearrange("(k p) n -> p k n", p=128))
            # x0 split in half so the very first matmul starts sooner
            nc.sync.dma_start(xT_r[:, 0, 0:QB], xT[0:128, 0:QB])
            nc.sync.dma_start(xT_r[:, 0, QB:S], xT[0:128, QB:S])
            for k in range(1, KCH):
                nc.sync.dma_start(xT_r[:, k, :], xT[k * 128:(k + 1) * 128, :])
            nc.sync.dma_start(wk_r[:], wk.rearrange("(k p) n -> p k n", p=128))
            nc.sync.dma_start(cos_sb[:], cosT[:, :])
            nc.sync.dma_start(sin_sb[:], sinT[:, :])
            nc.sync.dma_start(wv_r[:], wv.rearrange("(k p) n -> p k n", p=128))
            nc.sync.dma_start(wo_r[:], wo[:, :, :])
            nc.scalar.dma_start(bq_sb[:], bq[:, :])
            nc.scalar.dma_start(bk_sb[:], bk[:, :])

            ones_col = p0.tile([128, 1], BF16)
            nc.vector.memset(ones_col[:], 1.0)
            nc.vector.tensor_copy(
                v_r[:, :, :, DH:DH + 1],
                ones_col[:, None, None, :].broadcast_to([128, SKT, HL, 1]))
            # preload the exp ACT table while ACT is idle in phase A
            warm_in = p0.tile([1, 128], F32)
            warm = p0.tile([1, 128], F32)
            nc.vector.memset(warm_in[:], 1.0)
            nc.scalar.activation(warm[:], warm_in[:], EXP, scale=0.125)

            # ================= PHASE A =================
            with tc.tile_pool(name="ps_a", bufs=1, space="PSUM") as ps_a:
                # proj groups: (which weights, mt, q-block) -> rope dest
                groups = []
                for w_r, b_sb, dest in ((wq_r, bq_sb, qrope_r),
                                        (wk_r, bk_sb, krope_r)):
                    for mt in range(QT_TILES):
                        for qb_i in range(NQB):
                            groups.append((w_r, b_sb, dest, mt, qb_i))
                # sets of 3 share a k-outer pass; set 1 is all-wq (wk hasn't
                # arrived yet) and is paced by the x chunk DMAs
                order = [0, 1, 2, 3, 4, 5, 6, 7]
                sets = [order[0:3], order[3:6], order[6:8]]

                qb_by_group = {}
                for gset in sets:
                    pss = [ps_a.tile([128, QB], F32, tag="qkps", bufs=3,
                                     name=f"qkps{gi}") for gi in gset]
                    for k in range(KCH):
                        for ps, gi in zip(pss, gset):
                            _, _, _, mt, qb_i = groups[gi]
                            w_r = groups[gi][0]
                            q0 = qb_i * QB
                            for nq in range(QB // 512):
                                nc.tensor.matmul(
                                    ps[:, nq * 512:(nq + 1) * 512],
                                    w_r[:, k, mt * 128:(mt + 1) * 128],
                                    xT_r[:, k,
                                         q0 + nq * 512:q0 + (nq + 1) * 512],
                                    start=(k == 0), stop=(k == KCH - 1))
                    # only the PSUM evacuations go on the DVE FIFO here: they
                    # recycle the qkps ring for the next set with nothing
                    # (cos/sin, Pool) in front of them
                    for ps, gi in zip(pss, gset):
                        _, b_sb, _, mt, _ = groups[gi]
                        qb_r = pt.tile([128, QB], BF16, tag="qbr", bufs=8,
                                       name=f"qbr{gi}")
                        nc.vector.tensor_scalar(
                            out=qb_r[:], in0=ps[:],
                            scalar1=b_sb[:, mt:mt + 1],
                            scalar2=None, op0=ADD)
                        qb_by_group[gi] = qb_r

                # V projection: natural [S, depth]; bias-add evacuation on the
                # idle Pool engine so the DVE FIFO never gates the vps ring
                for sk in range(SKT):
                    vps = ps_a.tile([128, 512], F32, tag="vps", bufs=2)
                    for k in range(KCH):
                        nc.tensor.matmul(
                            vps[:, 0:DHL],
                            xT_r[:, k, sk * 128:(sk + 1) * 128],
                            wv_r[:, k, :],
                            start=(k == 0), stop=(k == KCH - 1))
                    nc.scalar.activation(
                        v_r[:, sk, :, 0:DH],
                        vps[:, 0:DHL].rearrange("p (h d) -> p h d", h=HL),
                        COPY)

                # rope combines, mt0 groups first (they gate phase B head 0)
                for gi in (0, 1, 4, 5, 2, 3, 6, 7):
                    _, _, dest, mt, qb_i = groups[gi]
                    q0 = qb_i * QB
                    qb_r = qb_by_group[gi]
                    rot = pt.tile([128, QB], BF16, tag="rot", bufs=2,
                                  name=f"rot{gi}")
                    nc.vector.stream_shuffle(rot[:], qb_r[:], SWAP_MASK)
                    t2 = pt.tile([128, QB], BF16, tag="t2", bufs=2,
                                 name=f"t2_{gi}")
                    nc.vector.tensor_mul(t2[:], rot[:], sin_sb[:, q0:q0 + QB])
                    t1 = pt.tile([128, QB], BF16, tag="t1", bufs=2,
                                 name=f"t1_{gi}")
                    nc.vector.tensor_mul(t1[:], qb_r[:],
                                         cos_sb[:, q0:q0 + QB])
                    nc.vector.tensor_add(dest[:, mt, q0:q0 + QB],
                                         t1[:], t2[:])

            # ================= PHASE B =================
            with tc.tile_pool(name="ps_b", bufs=1, space="PSUM") as ps_b:

                def oproj(qt, tail):
                    y_sb = pc_y.tile([128, DM], BF16, tag="ysb", bufs=6,
                                     name=f"ysb{qt}")
                    for ch in range(2):
                        ytag = "stps" if (tail and ch == 1) else "yring"
                        y_ps = ps_b.tile([128, 512], F32, tag=ytag,
                                         bufs=2, name=f"yps{qt}_{ch}")
                        for kc in range(QT_TILES):
                            nc.tensor.matmul(
                                y_ps[:],
                                outT_r[:, kc, qt * 128:(qt + 1) * 128],
                                wo_r[:, kc, ch * 512:(ch + 1) * 512],
                                start=(kc == 0), stop=(kc == QT_TILES - 1))
                        if tail and ch == 1:
                            nc.scalar.activation(
                                y_sb[:, ch * 512:(ch + 1) * 512], y_ps[:],
                                COPY)
                        else:
                            nc.vector.tensor_copy(
                                y_sb[:, ch * 512:(ch + 1) * 512], y_ps[:])
                    nc.sync.dma_start(y[qt * 128:(qt + 1) * 128, :], y_sb[:])

                def process_head(qb_i, h, q0, qbw, oproj_post,
                                 oproj_mid, last, sch=SCHRAUD_SKS):
                    nqb = qbw // 512
                    mt = h // 2
                    half = (h % 2) * DH
                    qt_h = qrope_r[half:half + DH, mt, :]
                    kt_h = krope_r[half:half + DH, mt, :]
                    pv_ps = ps_b.tile([DH + 1, QB], F32, tag="pvps",
                                      bufs=1, name=f"pv{qb_i}_{h}_{q0}")
                    deferred = []
                    first_pv = [True] * nqb

                    def pv_mm(exp_slice, sk, nq, last_mm):
                        nc.tensor.matmul(
                            pv_ps[:, nq * 512:(nq + 1) * 512],
                            v_r[:, sk, h, 0:DH + 1],
                            exp_slice,
                            start=first_pv[nq], stop=last_mm)
                        first_pv[nq] = False

                    for sk in range(SKT):
                        if sk in sch:
                            for nq in range(nqb):
                                sh_ps = ps_b.tile(
                                    [128, 512], F32, tag="yring", bufs=2,
                                    name=f"shps{sk}_{nq}")
                                nc.tensor.matmul(
                                    sh_ps[:],
                                    kt_h[:, sk * 128:(sk + 1) * 128],
                                    qt_h[:, q0 + nq * 512:
                                         q0 + (nq + 1) * 512],
                                    start=True, stop=True)
                                ei = pb_exp.tile([128, 512], U16,
                                                 tag="expi", bufs=10,
                                                 name=f"ei{sk}_{nq}")
                                nc.vector.tensor_scalar(
                                    out=ei[:], in0=sh_ps[:],
                                    scalar1=SCH_A, scalar2=SCH_B,
                                    op0=MULT, op1=ADD)
                                deferred.append(
                                    (ei[:, :].bitcast(BF16), sk, nq))
                        else:
                            st_ps = ps_b.tile([128, QB], F32, tag="stps",
                                              bufs=2)
                            for nq in range(nqb):
                                nc.tensor.matmul(
                                    st_ps[:, nq * 512:(nq + 1) * 512],
                                    kt_h[:, sk * 128:(sk + 1) * 128],
                                    qt_h[:, q0 + nq * 512:
                                         q0 + (nq + 1) * 512],
                                    start=True, stop=True)
                            ef = pb_exp.tile([128, QB], BF16,
                                             tag="expst")
                            nc.scalar.activation(ef[:, 0:qbw],
                                                 st_ps[:, 0:qbw], EXP,
                                                 scale=0.125)
                            for nq in range(nqb):
                                pv_mm(ef[:, nq * 512:(nq + 1) * 512],
                                      sk, nq, False)
                        for qt in oproj_mid.get(sk, ()):
                            oproj(qt, tail=False)
                    last_i = {}
                    for i, (_, _, nq) in enumerate(deferred):
                        last_i[nq] = i
                    for i, (exp_slice, sk, nq) in enumerate(deferred):
                        pv_mm(exp_slice, sk, nq, i == last_i[nq])

                    # evacuate PV raw (frees the bank in one copy);
                    # normalize off the critical path; the very last
                    # half normalizes straight from PSUM
                    if last:
                        pv_src = pv_ps
                    else:
                        pv_sb = pb_n.tile([DH + 1, QB], F32, tag="pvsb")
                        nc.vector.tensor_copy(pv_sb[:, 0:qbw],
                                              pv_ps[:, 0:qbw])
                        pv_src = pv_sb
                    den_t = pb_n.tile([1, QB], F32, tag="dent")
                    nc.vector.tensor_copy(den_t[:, 0:qbw],
                                          pv_src[DH:DH + 1, 0:qbw])
                    rec_b = pb_n.tile([DH, QB], F32, tag="recb")
                    nc.gpsimd.partition_broadcast(rec_b[:, 0:qbw],
                                                  den_t[:, 0:qbw])
                    nc.vector.reciprocal_approx_fast(
                        out=rec_b[:, 0:qbw], in_=rec_b[:, 0:qbw])
                    nc.vector.tensor_mul(
                        outT_r[half:half + DH, mt, q0:q0 + qbw],
                        pv_src[0:DH, 0:qbw], rec_b[:, 0:qbw])
                    for qt in oproj_post:
                        oproj(qt, tail=False)

                # qb0 heads, then qb1 heads with block-0 o-proj interleaved;
                # the last head is split into two 512-query halves so o-proj
                # tiles 8-11 hide inside the second half's compute
                # qb0 heads have no o-proj PE filler and run ACT-coupled:
                # a 5th fast-exp tile there drops ACT below the PE pace
                for h in range(HL):
                    process_head(0, h, 0, QB, (), {}, False,
                                 sch=(2, 5, 8, 11, 14))
                process_head(1, 0, QB, QB, (0, 1), {}, False)
                process_head(1, 1, QB, QB, (2, 3), {}, False)
                process_head(1, 2, QB, QB, (4, 5), {}, False)
                process_head(1, 3, QB, 512, (6, 7), {}, False)
                process_head(1, 3, QB + 512, 512, (8, 9, 10, 11), {}, True)
                for qt in range(12, 16):
                    oproj(qt, tail=True)

    nc.finalize()
    return nc


# host-side head-dim interleave: new[2i] = old[i], new[2i+1] = old[i+32]
_PERM64 = np.empty(DH, dtype=np.int64)
_PERM64[0::2] = np.arange(32)
_PERM64[1::2] = np.arange(32) + 32


def _rope_tables():
    """cos'/sin'' tables in the permuted layout, [128, S] (2 heads stacked).
    cos'[2i] = cos'[2i+1] = cos(ang_i); sin''[2i] = -sin(ang_i),
    sin''[2i+1] = +sin(ang_i) (rotation sign folded in)."""
    inv_freq = 1.0 / (10000.0 ** (np.arange(0, DH, 2, dtype=np.float32) / DH))
    ang = np.arange(S, dtype=np.float32)[:, None] * inv_freq[None, :]  # [S,32]
    cosp = np.empty((DH, S), dtype=np.float32)
    sinp = np.empty((DH, S), dtype=np.float32)
    cosp[0::2] = cosp[1::2] = np.cos(ang).T
    sinp[0::2] = -np.sin(ang).T
    sinp[1::2] = np.sin(ang).T
    cosT = np.ascontiguousarray(np.vstack([cosp, cosp]))
    sinT = np.ascontiguousarray(np.vstack([sinp, sinp]))
    return (sinT.astype(ml_dtypes.bfloat16), cosT.astype(ml_dtypes.bfloat16))


def _make_runner(nc):
    """Build a cached jitted SPMD executor (mirrors the multi-core tail of
    concourse.bass2jax.run_bass_via_pjrt so repeat calls skip recompiles)."""
    import jax
    import numpy as _np
    from jax.sharding import Mesh, PartitionSpec
    from jax.experimental.shard_map import shard_map
    from concourse import bass2jax, mybir as _mybir

    bass2jax.install_neuronx_cc_hook()

    partition_name = (
        nc.partition_id_tensor.name if nc.partition_id_tensor else None)
    in_names, out_names, out_avals, zero_shapes = [], [], [], []
    for alloc in nc.m.functions[0].allocations:
        if not isinstance(alloc, _mybir.MemoryLocationSet):
            continue
        name = alloc.memorylocations[0].name
        if alloc.kind == "ExternalInput":
            if name != partition_name:
                in_names.append(name)
        elif alloc.kind == "ExternalOutput":
            out_names.append(name)
            shape = tuple(alloc.tensor_shape)
            dtype = _mybir.dt.np(alloc.dtype)
            out_avals.append(jax.core.ShapedArray(shape, dtype))
            zero_shapes.append((shape, dtype))
    n_params = len(in_names)
    all_names = in_names + out_names
    if partition_name is not None:
        all_names = all_names + [partition_name]

    def _body(*args):
        operands = list(args)
        if partition_name is not None:
            operands.append(bass2jax.partition_id_tensor())
        outs = bass2jax._bass_exec_p.bind(
            *operands,
            out_avals=tuple(out_avals),
            in_names=tuple(all_names),
            out_names=tuple(out_names),
            lowering_input_output_aliases=(),
            sim_require_finite=True,
            sim_require_nnan=True,
            nc=nc,
        )
        return tuple(outs)

    devices = jax.devices()[:NCORES]
    mesh = Mesh(_np.asarray(devices), ("core",))
    n_outs = len(out_names)
    sharded = jax.jit(
        shard_map(
            _body, mesh=mesh,
            in_specs=(PartitionSpec("core"),) * (n_params + n_outs),
            out_specs=(PartitionSpec("core"),) * n_outs,
            check_rep=False,
        ),
        donate_argnums=tuple(range(n_params, n_params + n_outs)),
        keep_unused=True,
    )

    def run(in_maps):
        concat_in = [
            _np.concatenate([_np.asarray(m[name]) for m in in_maps], axis=0)
            for name in in_names
        ]
        concat_zeros = [
            _np.zeros((NCORES * s[0], *s[1:]), dt) for (s, dt) in zero_shapes
        ]
        out_arrs = sharded(*concat_in, *concat_zeros)
        return [
            {
                name: _np.asarray(out_arrs[i]).reshape(
                    NCORES, *out_avals[i].shape)[c]
                for i, name in enumerate(out_names)
            }
            for c in range(NCORES)
        ]

    return run


def _get_runner():
    if "runner" not in _CACHE:
        nc = _build_nc()
        _CACHE["nc"] = nc
        _CACHE["runner"] = _make_runner(nc)
    return _CACHE["runner"]


def make_in_maps(x, wq, bq, wk, bk, wv, bv, wo, bo):
    """Build the 8 per-core input dicts from full inputs."""
    x = np.asarray(x, dtype=np.float32)
    if "tables" not in _CACHE:
        _CACHE["tables"] = _rope_tables()
    sinT, cosT = _CACHE["tables"]
    bf = ml_dtypes.bfloat16
    # head-dim interleave permutation applied per head to wq/wk/bq/bk
    permH = (np.arange(H)[:, None] * DH + _PERM64[None, :]).reshape(-1)
    wq_p = np.asarray(wq, np.float32)[:, permH]
    wk_p = np.asarray(wk, np.float32)[:, permH]
    bq_p = np.asarray(bq, np.float32)[permH]
    bk_p = np.asarray(bk, np.float32)[permH]
    in_maps = []
    for c in range(NCORES):
        b, hg = divmod(c, HL)
        sl = slice(hg * DHL, (hg + 1) * DHL)
        in_maps.append({
            "xT": np.ascontiguousarray(x[b].T).astype(bf),
            "wq": np.ascontiguousarray(wq_p[:, sl]).astype(bf),
            "wk": np.ascontiguousarray(wk_p[:, sl]).astype(bf),
            "wv": np.ascontiguousarray(
                np.asarray(wv, np.float32)[:, sl]).astype(bf),
            "wo": np.ascontiguousarray(
                np.asarray(wo, np.float32)[sl, :].reshape(QT_TILES, 128, DM)
                .transpose(1, 0, 2)).astype(bf),
            "bq": np.ascontiguousarray(bq_p[sl].reshape(QT_TILES, 128).T),
            "bk": np.ascontiguousarray(bk_p[sl].reshape(QT_TILES, 128).T),
            "cosT": cosT,
            "sinT": sinT,
        })
    return in_maps


def kernel(x, wq, bq, wk, bk, wv, bv, wo, bo):
    runner = _get_runner()
    in_maps = make_in_maps(x, wq, bq, wk, bk, wv, bv, wo, bo)
    results = runner(in_maps)
    bo = np.asarray(bo, dtype=np.float32)
    # V bias is separable through the O-projection: (attn+bv)@wo =
    # attn@wo + bv@wo, so the kernel skips it and we add it here exactly
    corr = (np.asarray(bv, np.float32) @ np.asarray(wo, np.float32)
            + bo).astype(np.float32)
    out = np.empty((B, S, DM), dtype=np.float32)
    for b in range(B):
        acc = results[b * HL + 0]["y"].astype(np.float32, copy=True)
        for hg in range(1, HL):
            acc += results[b * HL + hg]["y"]
        out[b] = acc + corr[None, :]
    return out
